# revision 26
# baseline (speedup 1.0000x reference)
"""Despawn2D (8-level db-style DWT analysis + synthesis) on 8 Trainium2 cores.

Math: the reference's FFT circular convolutions with 4-tap filters reduce to
4-tap circular stencils (L = 8192 is a power of two, so the ReplicationPad is
a no-op).  Per level:

  analysis:  out[j]  = f0*a[2j] + f1*a[2j-1] + f2*a[2j-2] + f3*a[2j-3] (mod N)
             with f = h (approx) and f = g (detail), g = flip(h)*(+,-,+,-)

When the provided filter bank is orthogonal (it is for the db2 filters the
reference uses), synthesis(analysis(x)) == x exactly, so the "rec" output is
a copy of the input and only the analysis runs on the compute engines.  A
host-side fp64 check of the perfect-reconstruction property selects that fast
path; otherwise a full fp32 on-device synthesis variant is used.

Fast path datapath: fp16.  The correctness budget (2e-2 rel err) dwarfs the
~1e-3 error fp16 introduces, and fp16 halves the output HBM traffic and
quadruples PE matmul throughput vs fp32.  Work split per level: the PE
computes taps 0-2 of both filters as diagonal-weight matmuls accumulated in
PSUM (1 col/cycle at fp16; ScalarE seeds tap 0 via an activation write to
PSUM from level 2 on); the DVE applies tap 3 fused with the PSUM->SBUF
evacuation (scalar_tensor_tensor with in1=PSUM); ScalarE casts the fp32
input to fp16 once (which also feeds the rec output copy); GpSimd copies the
3-col circular halos.  Outputs are written to HBM as fp16, upcast on host.

Scheduling: input chunks load tail-first with the two row-tiles interleaved
(the rec writeback queues on the same DMA ring BEHIND the loads so input
gets the bandwidth first); level-groups are emitted in a wavefront order
matching earliest-possible-start so the in-order engine queues never block
ready work behind input-gated work; throwaway matmuls warm the PE p-state
while the input streams in.

Sharding: pure data parallel - 2048 rows / 8 cores = 256 rows/core,
processed as 2 tiles of 128 partitions x 8192.
"""

import numpy as np

LEVELS = 8
L = 8192
ROWS_TOTAL = 2048
N_CORES = 8
RPC = ROWS_TOTAL // N_CORES  # rows per core
P = 128  # SBUF partitions
NT = RPC // P  # tiles per core

PE_TAPS = 3       # taps 0..2 of each filter run on the tensor engine
NBLK = LEVELS * 2 * PE_TAPS  # diag weight blocks
CH_MM = 512       # matmul moving-operand column cap
CH_EV = 1024      # PSUM chunk width / evacuation op width

_nc_cache = {}

# detail block offsets inside a coeffs row: [d0 | d1 | ... | d7 | a8]
DOFF = []
_off = 0
for _lev in range(LEVELS):
    DOFF.append(_off)
    _off += L >> (_lev + 1)
AOFF = _off  # 8160
STAGE_LEV = 4  # levels >= this write into one staged SBUF block
STAGE_BASE = DOFF[STAGE_LEV]  # 7680
STAGE_W = L - STAGE_BASE  # 512


def _make_g(h):
    g = h[::-1].copy()
    g[1::2] *= -1.0
    return g


def _taps_array(scaling):
    """(LEVELS*8,) row: per level [h0..h3, g0..g3], tiled to (P, LEVELS*8)."""
    row = np.empty(LEVELS * 8, np.float32)
    for lev in range(LEVELS):
        h = scaling[lev].astype(np.float32)
        g = _make_g(h)
        row[lev * 8: lev * 8 + 4] = h
        row[lev * 8 + 4: lev * 8 + 8] = g
    return np.tile(row, (P, 1)).copy()


def _wts_array(scaling):
    """fp16 diagonal weight blocks for the PE taps: block b = lev*6 + f*3 + k
    holds diag(filter_f[k]) where f=0 is h (approx), f=1 is g (detail)."""
    w = np.zeros((P, NBLK * P), np.float16)
    idx = np.arange(P)
    for lev in range(LEVELS):
        h = scaling[lev].astype(np.float64)
        g = _make_g(h)
        for f, filt in enumerate((h, g)):
            for k in range(PE_TAPS):
                b = lev * 6 + f * 3 + k
                w[idx, b * P + idx] = np.float16(filt[k])
    return w


def _pr_is_identity(scaling):
    """fp64 host check: does synthesis(analysis(x)) == x for these filters?"""
    rng = np.random.default_rng(1234)
    n0 = 1 << (LEVELS + 2)
    x = rng.standard_normal((2, n0))
    a = x.copy()
    details = []
    for lev in range(LEVELS):
        h = scaling[lev].astype(np.float64)
        g = _make_g(h)
        N = a.shape[1]
        idx = (np.arange(N // 2)[:, None] * 2 - np.arange(4)[None, :]) % N
        d = (a[:, idx] * g).sum(-1)
        a = (a[:, idx] * h).sum(-1)
        details.append(d)
    r = a
    for lev in reversed(range(LEVELS)):
        h = scaling[lev].astype(np.float64)
        g = _make_g(h)
        d = details[lev]
        m = r.shape[1]
        out = np.empty((2, 2 * m))
        i = np.arange(m)
        out[:, 0::2] = (g[0] * d[:, i] + g[2] * d[:, (i + 1) % m]
                        + h[0] * r[:, i] + h[2] * r[:, (i + 1) % m])
        out[:, 1::2] = (g[1] * d[:, (i + 1) % m] + g[3] * d[:, (i + 2) % m]
                        + h[1] * r[:, (i + 1) % m] + h[3] * r[:, (i + 2) % m])
        r = out
    # scaling arrives as fp32, so an orthogonal filter bank reconstructs to
    # ~1e-8 (fp32 rounding of the filter constants), not fp64 precision.
    # Non-orthogonal filters give O(1) error, so 1e-6 separates cleanly.
    err = np.abs(r - x).max() / max(np.abs(x).max(), 1e-30)
    return err < 1e-6


def _build_fast():
    """Orthogonal shortcut: fp16 analysis only, rec = fp16 copy of x."""
    import concourse.bacc as bacc
    import concourse.mybir as mybir
    from concourse.tile import TileContext
    import contextlib

    f32 = mybir.dt.float32
    f16 = mybir.dt.float16
    Alu = mybir.AluOpType

    nc = bacc.Bacc()
    x = nc.dram_tensor("x", [RPC, L], f32, kind="ExternalInput")
    taps = nc.dram_tensor("taps", [P, LEVELS * 8], f32, kind="ExternalInput")
    wts = nc.dram_tensor("wts", [P, NBLK * P], f16, kind="ExternalInput")
    rec = nc.dram_tensor("rec", [RPC, L], f16, kind="ExternalOutput")
    coeffs = nc.dram_tensor("coeffs", [RPC, L], f16, kind="ExternalOutput")

    XCH = 1024            # input load/cast chunk
    NXCH = L // XCH       # 8

    with TileContext(nc) as tc:
        with contextlib.ExitStack() as ctx:
            cpool = ctx.enter_context(tc.tile_pool(name="consts", bufs=1))
            xpool = ctx.enter_context(tc.tile_pool(name="x32", bufs=8))
            xcpool = ctx.enter_context(tc.tile_pool(name="xc", bufs=2))
            apool = ctx.enter_context(tc.tile_pool(name="awork", bufs=2))
            dpool = ctx.enter_context(tc.tile_pool(name="dwork", bufs=2))
            ppool = ctx.enter_context(
                tc.tile_pool(name="psum", bufs=2, space="PSUM"))

            # constants first: the PE warm-up + level 0 need them, and they
            # must not queue behind the bulk input loads
            tp = cpool.tile([P, LEVELS * 8], f32)
            nc.scalar.dma_start(out=tp[:, :], in_=taps[:, :])
            wt = cpool.tile([P, NBLK * P], f16)
            nc.scalar.dma_start(out=wt[:, 0:6 * P], in_=wts[:, 0:6 * P])
            nc.scalar.dma_start(out=wt[:, 6 * P:], in_=wts[:, 6 * P:])

            # ---- input loads: interleave the two tiles, tail chunk of each
            # row first (it carries the circular wrap halo).  All on the sync
            # queue so they serialize AHEAD of the rec writebacks (input is
            # the critical resource early on).
            CHORD = tuple(range(NXCH - 1, -1, -1))
            x32s = {}
            for c in CHORD:
                for t in range(NT):
                    rows = slice(t * P, (t + 1) * P)
                    c0 = c * XCH
                    x32 = xpool.tile([P, XCH], f32, tag="x32", name="x32")
                    nc.sync.dma_start(out=x32[:, :], in_=x[rows, c0:c0 + XCH])
                    x32s[(t, c)] = x32

            def tap(lev, k):  # h taps (f32 scalars for DVE)
                c = lev * 8 + k
                return tp[:, c:c + 1]

            def gtap(lev, k):  # g taps
                c = lev * 8 + 4 + k
                return tp[:, c:c + 1]

            def wblk(lev, f, k):  # fp16 diag weight block
                b = lev * 6 + f * 3 + k
                return wt[:, b * P:(b + 1) * P]

            # warm up the PE p-state/activity ramp with throwaway matmuls on
            # the weights tile while the input is still loading
            for w in range(6):
                pw = ppool.tile([P, CH_EV], f32, tag="pa", name="pwarm")
                nc.tensor.matmul(pw[:, 0:512], wblk(0, 0, 0),
                                 wt[:, 0:512], start=True, stop=True)

            # ---- cast fp32 chunks to the fp16 ext buffer.  ext layout:
            # xc[:, i] == x[:, i-3] circularly, so the 3-col left halo [0:3)
            # mirrors the row tail.  Tile 0's two earliest chunks cast on DVE
            # (idle until the first evacuation), the rest on ScalarE; order
            # matches DMA arrival to avoid head-of-line blocking in the
            # in-order engine queues.
            xcs = []
            for t in range(NT):
                xc = xcpool.tile([P, L + 3], f16, tag="xc", name="xc")
                xcs.append(xc)
            for j, c in enumerate(CHORD):
                for t in range(NT):
                    c0 = c * XCH
                    xc = xcs[t]
                    dst = xc[:, 3 + c0:3 + c0 + XCH]
                    nc.scalar.copy(out=dst, in_=x32s[(t, c)][:, :])
                    if c == NXCH - 1:
                        # circular wrap halo, right behind the cast
                        nc.scalar.copy(out=xc[:, 0:3], in_=xc[:, L:L + 3])
            # rec output: fp16 copy of x.  On the sync queue BEHIND the x
            # loads: same HWDGE ring, so the writeback only consumes HBM
            # bandwidth once the input stream has drained.
            for j, c in enumerate(CHORD):
                for t in range(NT):
                    rows = slice(t * P, (t + 1) * P)
                    c0 = c * XCH
                    nc.sync.dma_start(out=rec[rows, c0:c0 + XCH],
                                      in_=xcs[t][:, 3 + c0:3 + c0 + XCH])

            # ---- analysis levels (levels outer, tiles inner) ----
            a_exts = list(xcs)
            stages = [None] * NT
            a_tiles = {}
            d_tiles = {}

            # wavefront schedule: emit each (level, group) at its earliest-
            # possible-start rank.  Groups run tail-first within a level; a
            # deeper level's tail group only needs the previous level's
            # mid groups, so it slots in BEFORE the input-gated head groups
            # of shallower levels (avoids head-of-line stalls in the
            # in-order engine queues while the last input chunks land).
            ngr_of = [max(1, (L >> (lev + 1)) // CH_EV) for lev in range(LEVELS)]
            sched = []
            for g in range(ngr_of[0] - 1, 0, -1):
                sched.append((0, g))
                for lev in range(1, LEVELS):
                    if ngr_of[lev] - 1 >= g:
                        sched.append((lev, g))
            sched.append((0, 0))
            for lev in range(1, LEVELS):
                sched.append((lev, 0))

            for si, (lev, g) in enumerate(sched):
                if (lev == 0 and g in (2, 1)) or (lev == 1 and g == 1):
                    for w in range(5):
                        pw = ppool.tile([P, CH_EV], f32, tag="pa",
                                        name="pwarm2")
                        nc.tensor.matmul(pw[:, 0:512], wblk(0, 0, 0),
                                         wt[:, 0:512], start=True, stop=True)
                for t in range(NT):
                    rows = slice(t * P, (t + 1) * P)
                    M = L >> (lev + 1)
                    last = lev == LEVELS - 1
                    ngr = ngr_of[lev]
                    glast = ngr - 1  # group producing a[M-3:M]

                    if g == glast:  # first visit of this (lev, t)
                        if lev < STAGE_LEV:
                            d_tiles[(lev, t)] = dpool.tile(
                                [P, M], f16, tag=f"d{lev}", name=f"d{lev}")
                        elif stages[t] is None:
                            stages[t] = dpool.tile([P, STAGE_W], f16,
                                                   tag="stage", name="stage")
                        if not last:
                            a_tiles[(lev, t)] = apool.tile(
                                [P, M + 3], f16, tag=f"a{lev}", name=f"a{lev}")
                    src = a_exts[t] if lev == 0 else a_tiles[(lev - 1, t)]
                    a_t = a_tiles.get((lev, t))
                    d_t = d_tiles.get((lev, t))

                    def dslice(c0, F):
                        if lev < STAGE_LEV:
                            return d_t[:, c0:c0 + F]
                        base = DOFF[lev] - STAGE_BASE
                        return stages[t][:, base + c0:base + c0 + F]

                    def aslice(c0, F):
                        if not last:
                            return a_t[:, 3 + c0:3 + c0 + F]
                        base = AOFF - STAGE_BASE
                        return stages[t][:, base + c0:base + c0 + F]

                    c0 = g * CH_EV
                    F = min(CH_EV, M - c0)
                    for f, (dst, tp0, tp3) in enumerate((
                            (aslice, tap(lev, 0), tap(lev, 3)),
                            (dslice, gtap(lev, 0), gtap(lev, 3)))):
                        ps = ppool.tile([P, CH_EV], f32,
                                        tag=("pa", "pd")[f],
                                        name=("pa", "pd")[f])
                        # tap 0: from level 2 on, ScalarE seeds the PSUM
                        # accumulator (its queue is past the input casts by
                        # then), saving a PE matmul per filter
                        if lev >= 2:
                            o = 3 + 2 * c0
                            nc.scalar.mul(ps[:, 0:F],
                                          src[:, o:o + 2 * F:2], tp0)
                            k0 = 1
                        else:
                            k0 = 0
                        # taps k0..2 on PE; tap-outer over the <=512-col
                        # sub-chunks so consecutive matmuls share weights
                        for k in range(k0, PE_TAPS):
                            for s0 in range(0, F, CH_MM):
                                Fs = min(CH_MM, F - s0)
                                o = 3 - k + 2 * (c0 + s0)
                                nc.tensor.matmul(
                                    ps[:, s0:s0 + Fs],
                                    wblk(lev, f, k),
                                    src[:, o:o + 2 * Fs:2],
                                    start=(k == 0),
                                    stop=(k == PE_TAPS - 1),
                                    skip_group_check=(k0 == 1),
                                )
                        # tap 3 fused with the PSUM evacuation on DVE
                        o = 2 * c0  # (3 - 3) + 2*c0
                        nc.vector.scalar_tensor_tensor(
                            out=dst(c0, F),
                            in0=src[:, o:o + 2 * F:2],
                            scalar=tp3,
                            in1=ps[:, 0:F],
                            op0=Alu.mult,
                            op1=Alu.add,
                        )
                        if f == 0 and not last and g == glast:
                            # left halo ext[0:3] = a[M-3:M], copied from the
                            # evac output on the (otherwise idle) gpsimd
                            nc.gpsimd.tensor_copy(
                                out=a_t[:, 0:3],
                                in_=a_t[:, M:M + 3])
                    # stream big detail chunks straight out (issued from
                    # the scalar queue; sync carries the input loads)
                    if lev < 2:
                        nc.sync.dma_start(
                            out=coeffs[rows,
                                       DOFF[lev] + c0:DOFF[lev] + c0 + F],
                            in_=dslice(c0, F))
                    if g == 0:  # last visit of this (lev, t)
                        if 2 <= lev < STAGE_LEV:
                            nc.scalar.dma_start(
                                out=coeffs[rows, DOFF[lev]:DOFF[lev] + M],
                                in_=d_t[:, 0:M])
                        if last:
                            nc.sync.dma_start(
                                out=coeffs[rows, STAGE_BASE:L],
                                in_=stages[t][:, :])

    nc.finalize()
    return nc


def _build_synth():
    """Fallback for non-orthogonal filter banks: full fp32 analysis +
    on-device synthesis (correct for arbitrary scaling)."""
    import concourse.bacc as bacc
    import concourse.mybir as mybir
    from concourse.tile import TileContext

    f32 = mybir.dt.float32
    Alu = mybir.AluOpType

    nc = bacc.Bacc()
    x = nc.dram_tensor("x", [RPC, L], f32, kind="ExternalInput")
    taps = nc.dram_tensor("taps", [P, LEVELS * 8], f32, kind="ExternalInput")
    rec = nc.dram_tensor("rec", [RPC, L], f32, kind="ExternalOutput")
    coeffs = nc.dram_tensor("coeffs", [RPC, L], f32, kind="ExternalOutput")

    with TileContext(nc) as tc:
        import contextlib
        with contextlib.ExitStack() as ctx:
            cpool = ctx.enter_context(tc.tile_pool(name="consts", bufs=1))
            xpool = ctx.enter_context(tc.tile_pool(name="xio", bufs=1))
            wpool = ctx.enter_context(tc.tile_pool(name="work", bufs=1))
            dpool = ctx.enter_context(tc.tile_pool(name="dwork", bufs=1))

            tp = cpool.tile([P, LEVELS * 8], f32)
            nc.sync.dma_start(out=tp[:, :], in_=taps[:, :])

            def tap(lev, k):
                c = lev * 8 + k
                return tp[:, c:c + 1]

            def gtap(lev, k):
                c = lev * 8 + 4 + k
                return tp[:, c:c + 1]

            Nh = L // 2
            xts = []
            for t in range(NT):
                rows = slice(t * P, (t + 1) * P)
                xlo = xpool.tile([P, 3 + Nh], f32, tag="xlo")
                xhi = xpool.tile([P, 3 + Nh], f32, tag="xhi")
                nc.sync.dma_start(out=xhi[:, 0:3 + Nh], in_=x[rows, Nh - 3:L])
                nc.sync.dma_start(out=xlo[:, 3:3 + Nh], in_=x[rows, 0:Nh])
                nc.vector.tensor_copy(out=xlo[:, 0:3], in_=xhi[:, Nh:Nh + 3])
                xts.append((xlo, xhi))

            # analysis, tile-sequential
            d_tiles_all = [[] for _ in range(NT)]
            a_lasts = [None] * NT
            a_exts = list(xts)
            for t in range(NT):
                for lev in range(LEVELS):
                    N = L >> lev
                    M = N >> 1
                    last = lev == LEVELS - 1
                    Mh = Nh // 2
                    if lev == 0:
                        halves = ((0, xts[t][0], Nh), (Mh, xts[t][1], Nh))
                    else:
                        halves = ((0, a_exts[t], N),)
                    if not last:
                        a_t = wpool.tile([P, M + 3], f32, tag=f"a{lev}")
                        a_main = a_t[:, 3:3 + M]
                    else:
                        a_t = wpool.tile([P, M + 2], f32, tag=f"a{lev}")
                        a_main = a_t[:, 0:M]
                    d_t = dpool.tile([P, M + 2], f32, tag=f"d{lev}")
                    d_main = d_t[:, 0:M]

                    for jb, srct, W in halves:
                        W2 = W >> 1
                        am = a_main[:, jb:jb + W2]
                        dm = d_main[:, jb:jb + W2]
                        nc.scalar.mul(am, srct[:, 3:3 + W:2], tap(lev, 0))
                        nc.scalar.mul(dm, srct[:, 3:3 + W:2], gtap(lev, 0))
                        for k in (1, 2, 3):
                            nc.vector.scalar_tensor_tensor(
                                out=am, in0=srct[:, 3 - k:3 - k + W:2],
                                scalar=tap(lev, k), in1=am,
                                op0=Alu.mult, op1=Alu.add)
                            nc.vector.scalar_tensor_tensor(
                                out=dm, in0=srct[:, 3 - k:3 - k + W:2],
                                scalar=gtap(lev, k), in1=dm,
                                op0=Alu.mult, op1=Alu.add)

                    doff = DOFF[lev]
                    nc.sync.dma_start(
                        out=coeffs[slice(t * P, (t + 1) * P), doff:doff + M],
                        in_=d_main)
                    if last:
                        nc.sync.dma_start(
                            out=coeffs[slice(t * P, (t + 1) * P),
                                       AOFF:AOFF + M],
                            in_=a_main)
                        # right halo for synthesis start
                        nc.vector.tensor_copy(out=a_t[:, M:M + 2],
                                              in_=a_t[:, 0:2])
                        a_lasts[t] = a_t
                    else:
                        nc.vector.tensor_copy(out=a_t[:, 0:3],
                                              in_=a_t[:, M:M + 3])
                    d_tiles_all[t].append(d_t)
                    a_exts[t] = a_t

            # synthesis
            for t in range(NT):
                rows = slice(t * P, (t + 1) * P)
                xlo, xhi = xts[t]
                d_tiles = d_tiles_all[t]
                r_ext = a_lasts[t]
                for lev in reversed(range(LEVELS)):
                    m = L >> (lev + 1)
                    d_t = d_tiles[lev]
                    nc.vector.tensor_copy(out=d_t[:, m:m + 2],
                                          in_=d_t[:, 0:2])
                    h4 = [tap(lev, k) for k in range(4)]
                    g4 = [gtap(lev, k) for k in range(4)]
                    if lev > 0:
                        o_t = wpool.tile([P, 2 * m + 2], f32, tag=f"r{lev}")
                        parts = ((0, m, o_t[:, 0:2 * m:2], o_t[:, 1:2 * m:2]),)
                    else:
                        mh = m // 2
                        parts = (
                            (0, mh, xlo[:, 3:3 + Nh:2], xlo[:, 4:3 + Nh:2]),
                            (mh, mh, xhi[:, 3:3 + Nh:2], xhi[:, 4:3 + Nh:2]),
                        )
                    for ib, w, ev, od in parts:
                        nc.vector.tensor_scalar_mul(ev, d_t[:, ib:ib + w],
                                                    g4[0])
                        for srct, s in (
                                (d_t[:, ib + 1:ib + w + 1], g4[2]),
                                (r_ext[:, ib:ib + w], h4[0]),
                                (r_ext[:, ib + 1:ib + w + 1], h4[2])):
                            nc.vector.scalar_tensor_tensor(
                                out=ev, in0=srct, scalar=s, in1=ev,
                                op0=Alu.mult, op1=Alu.add)
                        nc.vector.tensor_scalar_mul(od,
                                                    d_t[:, ib + 1:ib + w + 1],
                                                    g4[1])
                        for srct, s in (
                                (d_t[:, ib + 2:ib + w + 2], g4[3]),
                                (r_ext[:, ib + 1:ib + w + 1], h4[1]),
                                (r_ext[:, ib + 2:ib + w + 2], h4[3])):
                            nc.vector.scalar_tensor_tensor(
                                out=od, in0=srct, scalar=s, in1=od,
                                op0=Alu.mult, op1=Alu.add)
                    if lev > 0:
                        nc.vector.tensor_copy(out=o_t[:, 2 * m:2 * m + 2],
                                              in_=o_t[:, 0:2])
                        r_ext = o_t
                nc.sync.dma_start(out=rec[rows, 0:Nh], in_=xlo[:, 3:3 + Nh])
                nc.sync.dma_start(out=rec[rows, Nh:L], in_=xhi[:, 3:3 + Nh])

    nc.finalize()
    return nc


def _get_nc(variant):
    if variant not in _nc_cache:
        _nc_cache[variant] = (
            _build_fast() if variant == "fast" else _build_synth())
    return _nc_cache[variant]


def _variant(scaling):
    return "fast" if _pr_is_identity(scaling) else "synth"


def _in_maps(x, scaling, variant):
    taps = _taps_array(scaling)
    if variant == "fast":
        wts = _wts_array(scaling)
        return [
            {"x": np.ascontiguousarray(x[i * RPC:(i + 1) * RPC]),
             "taps": taps, "wts": wts}
            for i in range(N_CORES)
        ]
    return [
        {"x": np.ascontiguousarray(x[i * RPC:(i + 1) * RPC]), "taps": taps}
        for i in range(N_CORES)
    ]


def _gather(outs):
    rec = np.concatenate([outs[i]["rec"] for i in range(N_CORES)], axis=0)
    coeffs = np.concatenate([outs[i]["coeffs"] for i in range(N_CORES)],
                            axis=0)
    return rec.astype(np.float32), coeffs.astype(np.float32)


def kernel(x: np.ndarray, scaling: np.ndarray):
    from concourse.bass_utils import run_bass_kernel_spmd

    x = np.ascontiguousarray(np.asarray(x, np.float32))
    scaling = np.asarray(scaling, np.float32)
    assert x.shape == (ROWS_TOTAL, L), x.shape
    assert scaling.shape == (LEVELS, 4), scaling.shape

    variant = _variant(scaling)
    nc = _get_nc(variant)
    in_maps = _in_maps(x, scaling, variant)

    res = None
    last_err = None
    for attempt in range(3):
        try:
            res = run_bass_kernel_spmd(
                nc, in_maps, core_ids=list(range(N_CORES)))
            break
        except Exception as e:  # transient NRT device wedge: retry
            last_err = e
    if res is None:
        raise last_err
    return _gather(res.results)


# revision 27
# speedup vs baseline: 1.0217x; 1.0217x over previous
"""Despawn2D (8-level db-style DWT analysis + synthesis) on 8 Trainium2 cores.

Math: the reference's FFT circular convolutions with 4-tap filters reduce to
4-tap circular stencils (L = 8192 is a power of two, so the ReplicationPad is
a no-op).  Per level:

  analysis:  out[j]  = f0*a[2j] + f1*a[2j-1] + f2*a[2j-2] + f3*a[2j-3] (mod N)
             with f = h (approx) and f = g (detail), g = flip(h)*(+,-,+,-)

When the provided filter bank is orthogonal (it is for the db2 filters the
reference uses), synthesis(analysis(x)) == x exactly, so the "rec" output is
a copy of the input and only the analysis runs on the compute engines.  A
host-side fp64 check of the perfect-reconstruction property selects that fast
path; otherwise a full fp32 on-device synthesis variant is used.

Fast path datapath: fp16.  The correctness budget (2e-2 rel err) dwarfs the
~1e-3 error fp16 introduces, and fp16 halves the output HBM traffic and
quadruples PE matmul throughput vs fp32.  Work split per level: the PE
computes taps 0-2 of both filters as diagonal-weight matmuls accumulated in
PSUM (1 col/cycle at fp16; ScalarE seeds tap 0 via an activation write to
PSUM from level 2 on); the DVE applies tap 3 fused with the PSUM->SBUF
evacuation (scalar_tensor_tensor with in1=PSUM); ScalarE casts the fp32
input to fp16 once (which also feeds the rec output copy); GpSimd copies the
3-col circular halos.  Outputs are written to HBM as fp16, upcast on host.

Scheduling: input chunks load tail-first with the two row-tiles interleaved
(the rec writeback queues on the same DMA ring BEHIND the loads so input
gets the bandwidth first); level-groups are emitted in a wavefront order
matching earliest-possible-start so the in-order engine queues never block
ready work behind input-gated work; throwaway matmuls warm the PE p-state
while the input streams in.

Sharding: pure data parallel - 2048 rows / 8 cores = 256 rows/core,
processed as 2 tiles of 128 partitions x 8192.
"""

import numpy as np

LEVELS = 8
L = 8192
ROWS_TOTAL = 2048
N_CORES = 8
RPC = ROWS_TOTAL // N_CORES  # rows per core
P = 128  # SBUF partitions
NT = RPC // P  # tiles per core

PE_TAPS = 3       # taps 0..2 of each filter run on the tensor engine
NBLK = LEVELS * 2 * PE_TAPS  # diag weight blocks
CH_MM = 512       # matmul moving-operand column cap
CH_EV = 1024      # PSUM chunk width / evacuation op width

_nc_cache = {}

# detail block offsets inside a coeffs row: [d0 | d1 | ... | d7 | a8]
DOFF = []
_off = 0
for _lev in range(LEVELS):
    DOFF.append(_off)
    _off += L >> (_lev + 1)
AOFF = _off  # 8160
STAGE_LEV = 4  # levels >= this write into one staged SBUF block
STAGE_BASE = DOFF[STAGE_LEV]  # 7680
STAGE_W = L - STAGE_BASE  # 512


def _make_g(h):
    g = h[::-1].copy()
    g[1::2] *= -1.0
    return g


def _taps_array(scaling):
    """(LEVELS*8,) row: per level [h0..h3, g0..g3], tiled to (P, LEVELS*8)."""
    row = np.empty(LEVELS * 8, np.float32)
    for lev in range(LEVELS):
        h = scaling[lev].astype(np.float32)
        g = _make_g(h)
        row[lev * 8: lev * 8 + 4] = h
        row[lev * 8 + 4: lev * 8 + 8] = g
    return np.tile(row, (P, 1)).copy()


def _wts_array(scaling):
    """fp16 diagonal weight blocks for the PE taps: block b = lev*6 + f*3 + k
    holds diag(filter_f[k]) where f=0 is h (approx), f=1 is g (detail)."""
    w = np.zeros((P, NBLK * P), np.float16)
    idx = np.arange(P)
    for lev in range(LEVELS):
        h = scaling[lev].astype(np.float64)
        g = _make_g(h)
        for f, filt in enumerate((h, g)):
            for k in range(PE_TAPS):
                b = lev * 6 + f * 3 + k
                w[idx, b * P + idx] = np.float16(filt[k])
    return w


def _pr_is_identity(scaling):
    """fp64 host check: does synthesis(analysis(x)) == x for these filters?"""
    rng = np.random.default_rng(1234)
    n0 = 1 << (LEVELS + 2)
    x = rng.standard_normal((2, n0))
    a = x.copy()
    details = []
    for lev in range(LEVELS):
        h = scaling[lev].astype(np.float64)
        g = _make_g(h)
        N = a.shape[1]
        idx = (np.arange(N // 2)[:, None] * 2 - np.arange(4)[None, :]) % N
        d = (a[:, idx] * g).sum(-1)
        a = (a[:, idx] * h).sum(-1)
        details.append(d)
    r = a
    for lev in reversed(range(LEVELS)):
        h = scaling[lev].astype(np.float64)
        g = _make_g(h)
        d = details[lev]
        m = r.shape[1]
        out = np.empty((2, 2 * m))
        i = np.arange(m)
        out[:, 0::2] = (g[0] * d[:, i] + g[2] * d[:, (i + 1) % m]
                        + h[0] * r[:, i] + h[2] * r[:, (i + 1) % m])
        out[:, 1::2] = (g[1] * d[:, (i + 1) % m] + g[3] * d[:, (i + 2) % m]
                        + h[1] * r[:, (i + 1) % m] + h[3] * r[:, (i + 2) % m])
        r = out
    # scaling arrives as fp32, so an orthogonal filter bank reconstructs to
    # ~1e-8 (fp32 rounding of the filter constants), not fp64 precision.
    # Non-orthogonal filters give O(1) error, so 1e-6 separates cleanly.
    err = np.abs(r - x).max() / max(np.abs(x).max(), 1e-30)
    return err < 1e-6


def _build_fast():
    """Orthogonal shortcut: fp16 analysis only, rec = fp16 copy of x."""
    import concourse.bacc as bacc
    import concourse.mybir as mybir
    from concourse.tile import TileContext
    import contextlib

    f32 = mybir.dt.float32
    f16 = mybir.dt.float16
    Alu = mybir.AluOpType

    nc = bacc.Bacc()
    x = nc.dram_tensor("x", [RPC, L], f32, kind="ExternalInput")
    taps = nc.dram_tensor("taps", [P, LEVELS * 8], f32, kind="ExternalInput")
    wts = nc.dram_tensor("wts", [P, NBLK * P], f16, kind="ExternalInput")
    rec = nc.dram_tensor("rec", [RPC, L], f16, kind="ExternalOutput")
    coeffs = nc.dram_tensor("coeffs", [RPC, L], f16, kind="ExternalOutput")

    XCH = 1024            # input load/cast chunk
    NXCH = L // XCH       # 8

    with TileContext(nc) as tc:
        with contextlib.ExitStack() as ctx:
            cpool = ctx.enter_context(tc.tile_pool(name="consts", bufs=1))
            xpool = ctx.enter_context(tc.tile_pool(name="x32", bufs=6))
            xcpool = ctx.enter_context(tc.tile_pool(name="xc", bufs=2))
            apool = ctx.enter_context(tc.tile_pool(name="awork", bufs=2))
            dpool = ctx.enter_context(tc.tile_pool(name="dwork", bufs=2))
            ppool = ctx.enter_context(
                tc.tile_pool(name="psum", bufs=2, space="PSUM"))

            # constants first: the PE warm-up + level 0 need them, and they
            # must not queue behind the bulk input loads
            tp = cpool.tile([P, LEVELS * 8], f32)
            nc.scalar.dma_start(out=tp[:, :], in_=taps[:, :])
            wt = cpool.tile([P, NBLK * P], f16)
            nc.scalar.dma_start(out=wt[:, 0:6 * P], in_=wts[:, 0:6 * P])
            nc.scalar.dma_start(out=wt[:, 6 * P:], in_=wts[:, 6 * P:])

            # ---- input loads: interleave the two tiles, tail chunk of each
            # row first (it carries the circular wrap halo).  All on the sync
            # queue so they serialize AHEAD of the rec writebacks (input is
            # the critical resource early on).
            CHORD = tuple(range(NXCH - 1, -1, -1))
            x32s = {}
            for c in CHORD:
                for t in range(NT):
                    rows = slice(t * P, (t + 1) * P)
                    c0 = c * XCH
                    x32 = xpool.tile([P, XCH], f32, tag="x32", name="x32")
                    nc.sync.dma_start(out=x32[:, :], in_=x[rows, c0:c0 + XCH])
                    x32s[(t, c)] = x32

            def tap(lev, k):  # h taps (f32 scalars for DVE)
                c = lev * 8 + k
                return tp[:, c:c + 1]

            def gtap(lev, k):  # g taps
                c = lev * 8 + 4 + k
                return tp[:, c:c + 1]

            def wblk(lev, f, k):  # fp16 diag weight block
                b = lev * 6 + f * 3 + k
                return wt[:, b * P:(b + 1) * P]

            # warm up the PE p-state/activity ramp with throwaway matmuls on
            # the weights tile while the input is still loading
            for w in range(6):
                pw = ppool.tile([P, CH_EV], f32, tag="pa", name="pwarm")
                nc.tensor.matmul(pw[:, 0:512], wblk(0, 0, 0),
                                 wt[:, 0:512], start=True, stop=True)

            # ---- cast fp32 chunks to the fp16 ext buffer.  ext layout:
            # xc[:, i] == x[:, i-3] circularly, so the 3-col left halo [0:3)
            # mirrors the row tail.  Tile 0's two earliest chunks cast on DVE
            # (idle until the first evacuation), the rest on ScalarE; order
            # matches DMA arrival to avoid head-of-line blocking in the
            # in-order engine queues.
            xcs = []
            for t in range(NT):
                xc = xcpool.tile([P, L + 3], f16, tag="xc", name="xc")
                xcs.append(xc)
            for j, c in enumerate(CHORD):
                for t in range(NT):
                    c0 = c * XCH
                    xc = xcs[t]
                    dst = xc[:, 3 + c0:3 + c0 + XCH]
                    nc.scalar.copy(out=dst, in_=x32s[(t, c)][:, :])
                    if c == NXCH - 1:
                        # circular wrap halo, right behind the cast
                        nc.scalar.copy(out=xc[:, 0:3], in_=xc[:, L:L + 3])
            # rec output: fp16 copy of x.  On the sync queue BEHIND the x
            # loads: same HWDGE ring, so the writeback only consumes HBM
            # bandwidth once the input stream has drained.
            for j, c in enumerate(CHORD):
                for t in range(NT):
                    rows = slice(t * P, (t + 1) * P)
                    c0 = c * XCH
                    nc.sync.dma_start(out=rec[rows, c0:c0 + XCH],
                                      in_=xcs[t][:, 3 + c0:3 + c0 + XCH])

            # ---- analysis levels (levels outer, tiles inner) ----
            a_exts = list(xcs)
            stages = [None] * NT
            a_tiles = {}
            d_tiles = {}

            # wavefront schedule: emit each (level, group) at its earliest-
            # possible-start rank.  Groups run tail-first within a level; a
            # deeper level's tail group only needs the previous level's
            # mid groups, so it slots in BEFORE the input-gated head groups
            # of shallower levels (avoids head-of-line stalls in the
            # in-order engine queues while the last input chunks land).
            ngr_of = [max(1, (L >> (lev + 1)) // CH_EV) for lev in range(LEVELS)]
            sched = []
            for g in range(ngr_of[0] - 1, 0, -1):
                sched.append((0, g))
                for lev in range(1, LEVELS):
                    if ngr_of[lev] - 1 >= g:
                        sched.append((lev, g))
            sched.append((0, 0))
            for lev in range(1, LEVELS):
                sched.append((lev, 0))

            for si, (lev, g) in enumerate(sched):
                if lev == 0 and g in (2, 1):
                    for w in range(5):
                        pw = ppool.tile([P, CH_EV], f32, tag="pa",
                                        name="pwarm2")
                        nc.tensor.matmul(pw[:, 0:512], wblk(0, 0, 0),
                                         wt[:, 0:512], start=True, stop=True)
                for t in range(NT):
                    rows = slice(t * P, (t + 1) * P)
                    M = L >> (lev + 1)
                    last = lev == LEVELS - 1
                    ngr = ngr_of[lev]
                    glast = ngr - 1  # group producing a[M-3:M]

                    if g == glast:  # first visit of this (lev, t)
                        if lev < STAGE_LEV:
                            d_tiles[(lev, t)] = dpool.tile(
                                [P, M], f16, tag=f"d{lev}", name=f"d{lev}")
                        elif stages[t] is None:
                            stages[t] = dpool.tile([P, STAGE_W], f16,
                                                   tag="stage", name="stage")
                        if not last:
                            a_tiles[(lev, t)] = apool.tile(
                                [P, M + 3], f16, tag=f"a{lev}", name=f"a{lev}")
                    src = a_exts[t] if lev == 0 else a_tiles[(lev - 1, t)]
                    a_t = a_tiles.get((lev, t))
                    d_t = d_tiles.get((lev, t))

                    def dslice(c0, F):
                        if lev < STAGE_LEV:
                            return d_t[:, c0:c0 + F]
                        base = DOFF[lev] - STAGE_BASE
                        return stages[t][:, base + c0:base + c0 + F]

                    def aslice(c0, F):
                        if not last:
                            return a_t[:, 3 + c0:3 + c0 + F]
                        base = AOFF - STAGE_BASE
                        return stages[t][:, base + c0:base + c0 + F]

                    c0 = g * CH_EV
                    F = min(CH_EV, M - c0)
                    for f, (dst, tp0, tp3) in enumerate((
                            (aslice, tap(lev, 0), tap(lev, 3)),
                            (dslice, gtap(lev, 0), gtap(lev, 3)))):
                        ps = ppool.tile([P, CH_EV], f32,
                                        tag=("pa", "pd")[f],
                                        name=("pa", "pd")[f])
                        # tap 0: from level 2 on, ScalarE seeds the PSUM
                        # accumulator (its queue is past the input casts by
                        # then), saving a PE matmul per filter
                        if lev >= 2:
                            o = 3 + 2 * c0
                            nc.scalar.mul(ps[:, 0:F],
                                          src[:, o:o + 2 * F:2], tp0)
                            k0 = 1
                        else:
                            k0 = 0
                        # taps k0..2 on PE; tap-outer over the <=512-col
                        # sub-chunks so consecutive matmuls share weights
                        for k in range(k0, PE_TAPS):
                            for s0 in range(0, F, CH_MM):
                                Fs = min(CH_MM, F - s0)
                                o = 3 - k + 2 * (c0 + s0)
                                nc.tensor.matmul(
                                    ps[:, s0:s0 + Fs],
                                    wblk(lev, f, k),
                                    src[:, o:o + 2 * Fs:2],
                                    start=(k == 0),
                                    stop=(k == PE_TAPS - 1),
                                    skip_group_check=(k0 == 1),
                                )
                        # tap 3 fused with the PSUM evacuation on DVE
                        o = 2 * c0  # (3 - 3) + 2*c0
                        nc.vector.scalar_tensor_tensor(
                            out=dst(c0, F),
                            in0=src[:, o:o + 2 * F:2],
                            scalar=tp3,
                            in1=ps[:, 0:F],
                            op0=Alu.mult,
                            op1=Alu.add,
                        )
                        if f == 0 and not last and g == glast:
                            # left halo ext[0:3] = a[M-3:M], copied from the
                            # evac output on the (otherwise idle) gpsimd
                            nc.gpsimd.tensor_copy(
                                out=a_t[:, 0:3],
                                in_=a_t[:, M:M + 3])
                    # stream big detail chunks straight out (issued from
                    # the scalar queue; sync carries the input loads)
                    if lev < 2:
                        nc.sync.dma_start(
                            out=coeffs[rows,
                                       DOFF[lev] + c0:DOFF[lev] + c0 + F],
                            in_=dslice(c0, F))
                    if g == 0:  # last visit of this (lev, t)
                        if 2 <= lev < STAGE_LEV:
                            nc.scalar.dma_start(
                                out=coeffs[rows, DOFF[lev]:DOFF[lev] + M],
                                in_=d_t[:, 0:M])
                        if last:
                            nc.sync.dma_start(
                                out=coeffs[rows, STAGE_BASE:L],
                                in_=stages[t][:, :])

    nc.finalize()
    return nc


def _build_synth():
    """Fallback for non-orthogonal filter banks: full fp32 analysis +
    on-device synthesis (correct for arbitrary scaling)."""
    import concourse.bacc as bacc
    import concourse.mybir as mybir
    from concourse.tile import TileContext

    f32 = mybir.dt.float32
    Alu = mybir.AluOpType

    nc = bacc.Bacc()
    x = nc.dram_tensor("x", [RPC, L], f32, kind="ExternalInput")
    taps = nc.dram_tensor("taps", [P, LEVELS * 8], f32, kind="ExternalInput")
    rec = nc.dram_tensor("rec", [RPC, L], f32, kind="ExternalOutput")
    coeffs = nc.dram_tensor("coeffs", [RPC, L], f32, kind="ExternalOutput")

    with TileContext(nc) as tc:
        import contextlib
        with contextlib.ExitStack() as ctx:
            cpool = ctx.enter_context(tc.tile_pool(name="consts", bufs=1))
            xpool = ctx.enter_context(tc.tile_pool(name="xio", bufs=1))
            wpool = ctx.enter_context(tc.tile_pool(name="work", bufs=1))
            dpool = ctx.enter_context(tc.tile_pool(name="dwork", bufs=1))

            tp = cpool.tile([P, LEVELS * 8], f32)
            nc.sync.dma_start(out=tp[:, :], in_=taps[:, :])

            def tap(lev, k):
                c = lev * 8 + k
                return tp[:, c:c + 1]

            def gtap(lev, k):
                c = lev * 8 + 4 + k
                return tp[:, c:c + 1]

            Nh = L // 2
            xts = []
            for t in range(NT):
                rows = slice(t * P, (t + 1) * P)
                xlo = xpool.tile([P, 3 + Nh], f32, tag="xlo")
                xhi = xpool.tile([P, 3 + Nh], f32, tag="xhi")
                nc.sync.dma_start(out=xhi[:, 0:3 + Nh], in_=x[rows, Nh - 3:L])
                nc.sync.dma_start(out=xlo[:, 3:3 + Nh], in_=x[rows, 0:Nh])
                nc.vector.tensor_copy(out=xlo[:, 0:3], in_=xhi[:, Nh:Nh + 3])
                xts.append((xlo, xhi))

            # analysis, tile-sequential
            d_tiles_all = [[] for _ in range(NT)]
            a_lasts = [None] * NT
            a_exts = list(xts)
            for t in range(NT):
                for lev in range(LEVELS):
                    N = L >> lev
                    M = N >> 1
                    last = lev == LEVELS - 1
                    Mh = Nh // 2
                    if lev == 0:
                        halves = ((0, xts[t][0], Nh), (Mh, xts[t][1], Nh))
                    else:
                        halves = ((0, a_exts[t], N),)
                    if not last:
                        a_t = wpool.tile([P, M + 3], f32, tag=f"a{lev}")
                        a_main = a_t[:, 3:3 + M]
                    else:
                        a_t = wpool.tile([P, M + 2], f32, tag=f"a{lev}")
                        a_main = a_t[:, 0:M]
                    d_t = dpool.tile([P, M + 2], f32, tag=f"d{lev}")
                    d_main = d_t[:, 0:M]

                    for jb, srct, W in halves:
                        W2 = W >> 1
                        am = a_main[:, jb:jb + W2]
                        dm = d_main[:, jb:jb + W2]
                        nc.scalar.mul(am, srct[:, 3:3 + W:2], tap(lev, 0))
                        nc.scalar.mul(dm, srct[:, 3:3 + W:2], gtap(lev, 0))
                        for k in (1, 2, 3):
                            nc.vector.scalar_tensor_tensor(
                                out=am, in0=srct[:, 3 - k:3 - k + W:2],
                                scalar=tap(lev, k), in1=am,
                                op0=Alu.mult, op1=Alu.add)
                            nc.vector.scalar_tensor_tensor(
                                out=dm, in0=srct[:, 3 - k:3 - k + W:2],
                                scalar=gtap(lev, k), in1=dm,
                                op0=Alu.mult, op1=Alu.add)

                    doff = DOFF[lev]
                    nc.sync.dma_start(
                        out=coeffs[slice(t * P, (t + 1) * P), doff:doff + M],
                        in_=d_main)
                    if last:
                        nc.sync.dma_start(
                            out=coeffs[slice(t * P, (t + 1) * P),
                                       AOFF:AOFF + M],
                            in_=a_main)
                        # right halo for synthesis start
                        nc.vector.tensor_copy(out=a_t[:, M:M + 2],
                                              in_=a_t[:, 0:2])
                        a_lasts[t] = a_t
                    else:
                        nc.vector.tensor_copy(out=a_t[:, 0:3],
                                              in_=a_t[:, M:M + 3])
                    d_tiles_all[t].append(d_t)
                    a_exts[t] = a_t

            # synthesis
            for t in range(NT):
                rows = slice(t * P, (t + 1) * P)
                xlo, xhi = xts[t]
                d_tiles = d_tiles_all[t]
                r_ext = a_lasts[t]
                for lev in reversed(range(LEVELS)):
                    m = L >> (lev + 1)
                    d_t = d_tiles[lev]
                    nc.vector.tensor_copy(out=d_t[:, m:m + 2],
                                          in_=d_t[:, 0:2])
                    h4 = [tap(lev, k) for k in range(4)]
                    g4 = [gtap(lev, k) for k in range(4)]
                    if lev > 0:
                        o_t = wpool.tile([P, 2 * m + 2], f32, tag=f"r{lev}")
                        parts = ((0, m, o_t[:, 0:2 * m:2], o_t[:, 1:2 * m:2]),)
                    else:
                        mh = m // 2
                        parts = (
                            (0, mh, xlo[:, 3:3 + Nh:2], xlo[:, 4:3 + Nh:2]),
                            (mh, mh, xhi[:, 3:3 + Nh:2], xhi[:, 4:3 + Nh:2]),
                        )
                    for ib, w, ev, od in parts:
                        nc.vector.tensor_scalar_mul(ev, d_t[:, ib:ib + w],
                                                    g4[0])
                        for srct, s in (
                                (d_t[:, ib + 1:ib + w + 1], g4[2]),
                                (r_ext[:, ib:ib + w], h4[0]),
                                (r_ext[:, ib + 1:ib + w + 1], h4[2])):
                            nc.vector.scalar_tensor_tensor(
                                out=ev, in0=srct, scalar=s, in1=ev,
                                op0=Alu.mult, op1=Alu.add)
                        nc.vector.tensor_scalar_mul(od,
                                                    d_t[:, ib + 1:ib + w + 1],
                                                    g4[1])
                        for srct, s in (
                                (d_t[:, ib + 2:ib + w + 2], g4[3]),
                                (r_ext[:, ib + 1:ib + w + 1], h4[1]),
                                (r_ext[:, ib + 2:ib + w + 2], h4[3])):
                            nc.vector.scalar_tensor_tensor(
                                out=od, in0=srct, scalar=s, in1=od,
                                op0=Alu.mult, op1=Alu.add)
                    if lev > 0:
                        nc.vector.tensor_copy(out=o_t[:, 2 * m:2 * m + 2],
                                              in_=o_t[:, 0:2])
                        r_ext = o_t
                nc.sync.dma_start(out=rec[rows, 0:Nh], in_=xlo[:, 3:3 + Nh])
                nc.sync.dma_start(out=rec[rows, Nh:L], in_=xhi[:, 3:3 + Nh])

    nc.finalize()
    return nc


def _get_nc(variant):
    if variant not in _nc_cache:
        _nc_cache[variant] = (
            _build_fast() if variant == "fast" else _build_synth())
    return _nc_cache[variant]


def _variant(scaling):
    return "fast" if _pr_is_identity(scaling) else "synth"


def _in_maps(x, scaling, variant):
    taps = _taps_array(scaling)
    if variant == "fast":
        wts = _wts_array(scaling)
        return [
            {"x": np.ascontiguousarray(x[i * RPC:(i + 1) * RPC]),
             "taps": taps, "wts": wts}
            for i in range(N_CORES)
        ]
    return [
        {"x": np.ascontiguousarray(x[i * RPC:(i + 1) * RPC]), "taps": taps}
        for i in range(N_CORES)
    ]


def _gather(outs):
    rec = np.concatenate([outs[i]["rec"] for i in range(N_CORES)], axis=0)
    coeffs = np.concatenate([outs[i]["coeffs"] for i in range(N_CORES)],
                            axis=0)
    return rec.astype(np.float32), coeffs.astype(np.float32)


def kernel(x: np.ndarray, scaling: np.ndarray):
    from concourse.bass_utils import run_bass_kernel_spmd

    x = np.ascontiguousarray(np.asarray(x, np.float32))
    scaling = np.asarray(scaling, np.float32)
    assert x.shape == (ROWS_TOTAL, L), x.shape
    assert scaling.shape == (LEVELS, 4), scaling.shape

    variant = _variant(scaling)
    nc = _get_nc(variant)
    in_maps = _in_maps(x, scaling, variant)

    res = None
    last_err = None
    for attempt in range(3):
        try:
            res = run_bass_kernel_spmd(
                nc, in_maps, core_ids=list(range(N_CORES)))
            break
        except Exception as e:  # transient NRT device wedge: retry
            last_err = e
    if res is None:
        raise last_err
    return _gather(res.results)
